# revision 21
# baseline (speedup 1.0000x reference)
"""Trainium2 Bass kernel for a decoder-only transformer forward pass.

Reference semantics (see problem): the layer loop never feeds its output
back, so only the LAST layer's block matters:
    h   = E[x] + pos_encoding                         [B, S, D]
    Q/K/V = h @ Wq/k/v + b                            (last layer)
    att = softmax(causal(QK^T/sqrt(dk))) @ V
    h2  = att @ Wo + bo
    dec = tanh(h2 @ W1 + b1)                          [B, S, M]
    out = dec @ Wout + bout                           [B, S, OMEGA]

Sharding: 8 cores; core c handles batch b=c//4, query rows
[q0, q0+512), q0=(c%4)*512.  Every core computes K/V for its whole batch
(redundant x4) and its own 512-row slice of everything else, including
the dominant output-head matmul.  No collectives needed.

Device dataflow keeps activations TRANSPOSED (feature dim on partitions)
until the output head, so no on-device transposes are ever needed:
  hT -> QT/KT (lhsT = W in native layout), V (lhsT = hT)
  ST[k,q] = K.Q per head, P = exp(ST/sqrt(dk)) * causal_mask
  attT_unnorm[d',q] via lhsT=V; an appended ones-column in V yields the
  softmax denominator for free; normalize via reciprocal + ones-matmul
  partition-broadcast.
  h2T = Wo-matmul, decT = tanh(W1-matmul), logits = decT^T-matmul (the
  head flips back to q-on-partitions so the output DMA is contiguous).

All matmul operands are bf16 (fp32 PSUM accumulation).
"""

import sys

sys.path.insert(0, "/opt/trn_rl_repo")

import numpy as np
import ml_dtypes

import concourse.bass as bass  # noqa: F401
import concourse.mybir as mybir
import concourse.tile as tile
from concourse import bacc
from concourse.bass_utils import run_bass_kernel_spmd

P = 128
BF16 = mybir.dt.bfloat16
F32 = mybir.dt.float32


class Cfg:
    def __init__(self, S=2048, D=1024, DM=4096, V=16000, SQ=512, DK=64):
        self.S, self.D, self.DM, self.V, self.SQ, self.DK = S, D, DM, V, SQ, DK
        self.H = D // DK          # heads
        self.DC = D // P          # d chunks
        self.MC = DM // P         # ffn chunks
        self.KC = S // P          # key chunks
        self.QC = SQ // P         # q chunks


FULL = Cfg()


def _blocks(total, w=512):
    out = []
    v0 = 0
    while v0 < total:
        out.append((v0, min(w, total - v0)))
        v0 += w
    return out


def build_nc(cfg=FULL, debug=False, phases=4):
    c = cfg
    nc = bacc.Bacc("TRN2", target_bir_lowering=False, debug=debug)

    # weights grouped by OUTPUT chunk so they can be streamed per chunk:
    # wq[ki, out_ch, kc, kj] = Wq[kc*P+ki, out_ch*P+kj]
    hT = nc.dram_tensor("hT", [P, c.DC, c.S], BF16, kind="ExternalInput")
    hTq = nc.dram_tensor("hTq", [P, c.DC, c.SQ], BF16, kind="ExternalInput")
    wq = nc.dram_tensor("wq", [P, c.DC, c.DC, P], BF16, kind="ExternalInput")
    wk = nc.dram_tensor("wk", [P, c.DC, c.DC, P], BF16, kind="ExternalInput")
    wv = nc.dram_tensor("wv", [P, c.DC, c.D], BF16, kind="ExternalInput")
    wo = nc.dram_tensor("wo", [P, c.DC, c.D], BF16, kind="ExternalInput")
    w1 = nc.dram_tensor("w1", [P, c.MC, c.DC, P], BF16, kind="ExternalInput")
    wout = nc.dram_tensor("wout", [P, c.MC, c.V], BF16, kind="ExternalInput")
    mask = nc.dram_tensor("mask", [P, c.KC, c.SQ], BF16, kind="ExternalInput")
    bqp = nc.dram_tensor("bqp", [P, c.DC], F32, kind="ExternalInput")
    bkp = nc.dram_tensor("bkp", [P, c.DC], F32, kind="ExternalInput")
    bop = nc.dram_tensor("bop", [P, c.DC], F32, kind="ExternalInput")
    b1p = nc.dram_tensor("b1p", [P, c.MC], F32, kind="ExternalInput")
    bvr = nc.dram_tensor("bvr", [1, c.D], BF16, kind="ExternalInput")
    boutr = nc.dram_tensor("boutr", [1, c.V], BF16, kind="ExternalInput")
    out = nc.dram_tensor("out", [c.SQ, c.V], F32, kind="ExternalOutput")

    scale = 1.0 / np.sqrt(np.float32(c.DK))

    with tile.TileContext(nc) as tc:
        with (
            tc.tile_pool(name="const", bufs=1) as const,
            tc.tile_pool(name="persistB", bufs=1) as persistB,
        ):
            ones = const.tile([1, P], BF16, tag="ones")
            nc.any.memset(ones[:], 1.0)
            bq_sb = const.tile([P, c.DC], F32, tag="bq")
            bk_sb = const.tile([P, c.DC], F32, tag="bk")
            bo_sb = const.tile([P, c.DC], F32, tag="bo")
            b1_sb = const.tile([P, c.MC], F32, tag="b1")
            bv_sb = const.tile([1, c.D], BF16, tag="bv")
            nc.sync.dma_start(bq_sb[:], bqp[:])
            nc.sync.dma_start(bk_sb[:], bkp[:])
            nc.sync.dma_start(bo_sb[:], bop[:])
            nc.sync.dma_start(b1_sb[:], b1p[:])
            nc.sync.dma_start(bv_sb[:], bvr[:])

            attn_sb = persistB.tile([P, c.DC, c.SQ], BF16, tag="attn")
            dec_sb = persistB.tile([P, c.MC, c.SQ], BF16, tag="dec")

            with tc.tile_pool(name="persistA", bufs=1) as persistA:
                qt_sb = persistA.tile([P, c.DC, c.SQ], BF16, tag="qt")
                kt_sb = persistA.tile([P, c.DC, c.S], BF16, tag="kt")
                # V with an appended ones-column per head: [P, H, DK+1]
                v_sb = [
                    persistA.tile([P, c.H, c.DK + 1], BF16, tag=f"v{kk}",
                                  name=f"v{kk}")
                    for kk in range(c.KC)
                ]

                # ---------------- phase 1: projections ----------------
                with (
                    tc.tile_pool(name="wstream", bufs=3) as wstream,
                    tc.tile_pool(name="wvpool", bufs=1) as wvpool,
                    tc.tile_pool(name="hpool", bufs=1) as hpool,
                    tc.tile_pool(name="ps1", bufs=4, space="PSUM") as ps1,
                ):
                    wv_sb = wvpool.tile([P, c.DC, c.D], BF16, tag="wv")
                    hT_sb = hpool.tile([P, c.DC, c.S], BF16, tag="hT")
                    hTq_sb = hpool.tile([P, c.DC, c.SQ], BF16, tag="hTq")
                    # hTq gates the very first matmuls — issue it first
                    for ch in range(c.DC):
                        nc.sync.dma_start(hTq_sb[:, ch, :], hTq[:, ch, :])
                    for ch in range(c.DC):
                        nc.sync.dma_start(hT_sb[:, ch, :], hT[:, ch, :])
                    for ch in range(c.DC):
                        nc.sync.dma_start(wv_sb[:, ch, :], wv[:, ch, :])

                    # QT[d',q] = sum_d Wq[d,d'] hT[d,q]  (+bq)
                    for ch in range(c.DC):
                        wt = wstream.tile([P, c.DC, P], BF16, tag="wt")
                        nc.sync.dma_start(wt[:], wq[:, ch])
                        ps = ps1.tile([P, 512], F32, tag="ps1t")
                        for kc in range(c.DC):
                            nc.tensor.matmul(
                                ps[:, : c.SQ],
                                wt[:, kc, :],
                                hTq_sb[:, kc, :],
                                start=(kc == 0),
                                stop=(kc == c.DC - 1),
                            )
                        nc.vector.tensor_scalar_add(
                            qt_sb[:, ch, :], ps[:, : c.SQ],
                            bq_sb[:, ch : ch + 1],
                        )

                    # KT[d',k] over all S keys (+bk)
                    for ch in range(c.DC):
                        wt = wstream.tile([P, c.DC, P], BF16, tag="wt")
                        nc.sync.dma_start(wt[:], wk[:, ch])
                        for (n0, W) in _blocks(c.S):
                            ps = ps1.tile([P, 512], F32, tag="ps1t")
                            for kc in range(c.DC):
                                nc.tensor.matmul(
                                    ps[:, :W],
                                    wt[:, kc, :],
                                    hT_sb[:, kc, n0 : n0 + W],
                                    start=(kc == 0),
                                    stop=(kc == c.DC - 1),
                                )
                            nc.vector.tensor_scalar_add(
                                kt_sb[:, ch, n0 : n0 + W],
                                ps[:, :W],
                                bk_sb[:, ch : ch + 1],
                            )

                    # V[k,d'] native; lhsT = hT key-chunk, rhs = Wv
                    # (+bv via ones-row matmul on the free axis)
                    for kk in range(c.KC):
                        nc.any.memset(v_sb[kk][:, :, c.DK : c.DK + 1], 1.0)
                        for (d0, W) in _blocks(c.D):
                            ps = ps1.tile([P, 512], F32, tag="ps1t")
                            for kc in range(c.DC):
                                nc.tensor.matmul(
                                    ps[:, :W],
                                    hT_sb[:, kc, kk * P : (kk + 1) * P],
                                    wv_sb[:, kc, d0 : d0 + W],
                                    start=(kc == 0),
                                    stop=False,
                                )
                            nc.tensor.matmul(
                                ps[:, :W],
                                ones[0:1, :],
                                bv_sb[0:1, d0 : d0 + W],
                                start=False,
                                stop=True,
                            )
                            h0 = d0 // c.DK
                            h1 = (d0 + W) // c.DK
                            nc.vector.tensor_copy(
                                v_sb[kk][:, h0:h1, 0 : c.DK],
                                ps[:, :W].rearrange("p (h d) -> p h d",
                                                    d=c.DK),
                            )

                # ---------------- phase 2: attention ----------------
                with (
                    tc.tile_pool(name="maskp", bufs=1) as maskp,
                    tc.tile_pool(name="ppool", bufs=8) as ppool,
                    tc.tile_pool(name="npool", bufs=4) as npool,
                    tc.tile_pool(name="st_ps", bufs=5, space="PSUM") as st_ps,
                    tc.tile_pool(name="at_ps", bufs=2, space="PSUM") as at_ps,
                    tc.tile_pool(name="rb_ps", bufs=1, space="PSUM") as rb_ps,
                ):
                    mask_sb = maskp.tile([P, c.KC, c.SQ], BF16, tag="mask")
                    for kk in range(c.KC):
                        nc.sync.dma_start(mask_sb[:, kk, :], mask[:, kk, :])

                    for h in range(c.H if phases >= 2 else 0):
                        hp = 64 * (h % 2)
                        ch = h // 2
                        at = at_ps.tile([P, c.SQ], F32, tag="at")
                        p_tiles = []
                        for kk in range(c.KC):
                            st = st_ps.tile([P, c.SQ], F32, tag="st")
                            nc.tensor.matmul(
                                st[:],
                                kt_sb[hp : hp + c.DK, ch,
                                      kk * P : (kk + 1) * P],
                                qt_sb[hp : hp + c.DK, ch, :],
                                start=True,
                                stop=True,
                            )
                            p = ppool.tile([P, c.SQ], BF16, tag="p")
                            nc.scalar.activation(
                                p[:], st[:],
                                mybir.ActivationFunctionType.Exp,
                                scale=float(scale),
                            )
                            nc.vector.tensor_tensor(
                                p[:], p[:], mask_sb[:, kk, :],
                                mybir.AluOpType.mult,
                            )
                            p_tiles.append(p)
                        for kk in range(c.KC):
                            nc.tensor.matmul(
                                at[0 : c.DK + 1, :],
                                v_sb[kk][:, h, :],
                                p_tiles[kk][:],
                                start=(kk == 0),
                                stop=(kk == c.KC - 1),
                            )
                        # normalize by the ones-column sum (row DK of at)
                        r32 = npool.tile([1, c.SQ], F32, tag="r32")
                        nc.vector.reciprocal(r32[:], at[c.DK : c.DK + 1, :])
                        rbf = npool.tile([1, c.SQ], BF16, tag="rbf")
                        nc.vector.tensor_copy(rbf[:], r32[:])
                        rb = rb_ps.tile([c.DK, c.SQ], F32, tag="rb")
                        nc.tensor.matmul(
                            rb[:], ones[0:1, 0 : c.DK], rbf[:],
                            start=True, stop=True,
                        )
                        # rb to SBUF (ACT, idle-ish) so the normalize mult can
                        # read the attT accumulator straight from PSUM on DVE
                        rb_sb = npool.tile([c.DK, c.SQ], F32, tag="rb_sb")
                        nc.scalar.activation(
                            rb_sb[:], rb[:],
                            mybir.ActivationFunctionType.Copy,
                        )
                        nc.vector.tensor_tensor(
                            attn_sb[hp : hp + c.DK, ch, :],
                            at[0 : c.DK, :],
                            rb_sb[:],
                            mybir.AluOpType.mult,
                        )

            # ------- phases 3+4 share one scope: Wo + FFN + output head ----
            # (lets the first Wout slab DMAs and head matmuls overlap the
            # FFN tail instead of waiting for a pool boundary)
            with (
                tc.tile_pool(name="wopool", bufs=1) as wopool,
                tc.tile_pool(name="w1stream", bufs=3) as w1stream,
                tc.tile_pool(name="h2pool", bufs=1) as h2pool,
                tc.tile_pool(name="ps3", bufs=3, space="PSUM") as ps3,
                tc.tile_pool(name="wout_p", bufs=3) as wout_p,
                tc.tile_pool(name="bout_p", bufs=2) as bout_p,
                tc.tile_pool(name="out_p", bufs=4) as out_p,
                tc.tile_pool(name="ps4", bufs=3, space="PSUM") as ps4,
            ):
                wo_sb = wopool.tile([P, c.DC, c.D], BF16, tag="wo")
                for ch in range(c.DC):
                    nc.sync.dma_start(wo_sb[:, ch, :], wo[:, ch, :])
                h2_sb = h2pool.tile([P, c.DC, c.SQ], BF16, tag="h2")
                for ch in range(c.DC if phases >= 3 else 0):
                    ps = ps3.tile([P, 512], F32, tag="ps3t")
                    for kc in range(c.DC):
                        nc.tensor.matmul(
                            ps[:, : c.SQ],
                            wo_sb[:, kc, ch * P : (ch + 1) * P],
                            attn_sb[:, kc, :],
                            start=(kc == 0),
                            stop=(kc == c.DC - 1),
                        )
                    nc.vector.tensor_scalar_add(
                        h2_sb[:, ch, :], ps[:, : c.SQ], bo_sb[:, ch : ch + 1]
                    )
                for mch in range(c.MC if phases >= 3 else 0):
                    wt = w1stream.tile([P, c.DC, P], BF16, tag="w1t")
                    nc.sync.dma_start(wt[:], w1[:, mch])
                    ps = ps3.tile([P, 512], F32, tag="ps3t")
                    for kc in range(c.DC):
                        nc.tensor.matmul(
                            ps[:, : c.SQ],
                            wt[:, kc, :],
                            h2_sb[:, kc, :],
                            start=(kc == 0),
                            stop=(kc == c.DC - 1),
                        )
                    nc.scalar.activation(
                        dec_sb[:, mch, :], ps[:, : c.SQ],
                        mybir.ActivationFunctionType.Tanh,
                        bias=b1_sb[:, mch : mch + 1],
                        scale=1.0,
                    )

                # ---------------- phase 4: output head ----------------
                for (v0, W) in (_blocks(c.V) if phases >= 4 else []):
                    wt = wout_p.tile([P, c.MC, 512], BF16, tag="wt4")
                    step = max(1, c.MC // 4)
                    for m0 in range(0, c.MC, step):
                        m1 = min(c.MC, m0 + step)
                        nc.sync.dma_start(
                            wt[:, m0:m1, :W], wout[:, m0:m1, v0 : v0 + W]
                        )
                    bt = bout_p.tile([1, 512], BF16, tag="bt")
                    nc.sync.dma_start(bt[0:1, :W], boutr[0:1, v0 : v0 + W])
                    # broadcast bout across partitions once per v-block;
                    # the bias is then added during psum eviction on DVE
                    bb = ps4.tile([P, 512], F32, tag="bb", bufs=1)
                    nc.tensor.matmul(
                        bb[:, :W], ones[0:1, :], bt[0:1, :W],
                        start=True, stop=True,
                    )
                    bb_sb = bout_p.tile([P, 512], BF16, tag="bb_sb")
                    nc.scalar.activation(
                        bb_sb[:, :W], bb[:, :W],
                        mybir.ActivationFunctionType.Copy,
                    )
                    for qc in range(c.QC):
                        ps = ps4.tile([P, 512], F32, tag="ps4t")
                        for mch in range(c.MC):
                            nc.tensor.matmul(
                                ps[:, :W],
                                dec_sb[:, mch, qc * P : (qc + 1) * P],
                                wt[:, mch, :W],
                                start=(mch == 0),
                                stop=(mch == c.MC - 1),
                            )
                        ot = out_p.tile([P, 512], F32, tag="ot")
                        nc.vector.tensor_tensor(
                            ot[:, :W], ps[:, :W], bb_sb[:, :W],
                            mybir.AluOpType.add,
                        )
                        nc.sync.dma_start(
                            out[qc * P : (qc + 1) * P, v0 : v0 + W],
                            ot[:, :W],
                        )

    nc.compile()
    return nc


# ---------------------------------------------------------------------------
# host side
# ---------------------------------------------------------------------------

def _pos_encoding(seq_len, d):
    pos = np.arange(seq_len, dtype=np.float32)[:, None]
    div = np.exp(
        np.arange(0, d, 2, dtype=np.float32) * (-np.log(10000.0) / d)
    )
    pe = np.zeros((seq_len, d), dtype=np.float32)
    pe[:, 0::2] = np.sin(pos * div)
    pe[:, 1::2] = np.cos(pos * div)
    return pe


def _chunked(a, pdim_chunks):
    """[N, F] -> [128, N//128, F] with row n = ko*128+ki -> [ki, ko, f]."""
    n, f = a.shape
    return np.ascontiguousarray(
        a.reshape(pdim_chunks, P, f).transpose(1, 0, 2)
    )


def _grouped(a, kchunks, ochunks):
    """[K, O] -> [128, O//128, K//128, 128]: [ki, oc, kc, kj]."""
    k, o = a.shape
    return np.ascontiguousarray(
        a.reshape(kchunks, P, ochunks, P).transpose(1, 2, 0, 3)
    )


_NC_CACHE = {}


def _get_nc(cfg=FULL):
    key = (cfg.S, cfg.D, cfg.DM, cfg.V, cfg.SQ)
    if key not in _NC_CACHE:
        _NC_CACHE[key] = build_nc(cfg)
    return _NC_CACHE[key]


def make_in_maps(x, E, Wq, bq, Wk, bk, Wv, bv, Wo, bo, W1, b1, Wout, bout,
                 cfg=FULL, n_cores=8):
    c = cfg
    bf = ml_dtypes.bfloat16
    x = np.asarray(x)
    E = np.asarray(E, dtype=np.float32)
    B = x.shape[0]
    h = E[x] + _pos_encoding(x.shape[1], E.shape[1])[None]

    wq_a = _grouped(np.asarray(Wq[-1]).astype(bf), c.DC, c.DC)
    wk_a = _grouped(np.asarray(Wk[-1]).astype(bf), c.DC, c.DC)
    wv_a = _chunked(np.asarray(Wv[-1]).astype(bf), c.DC)
    wo_a = _chunked(np.asarray(Wo[-1]).astype(bf), c.DC)
    w1_a = _grouped(np.asarray(W1[-1]).astype(bf), c.DC, c.MC)
    wout_a = _chunked(np.asarray(Wout).astype(bf), c.MC)
    f32 = np.float32
    bq_a = np.ascontiguousarray(np.asarray(bq[-1]).reshape(c.DC, P).T).astype(f32)
    bk_a = np.ascontiguousarray(np.asarray(bk[-1]).reshape(c.DC, P).T).astype(f32)
    bo_a = np.ascontiguousarray(np.asarray(bo[-1]).reshape(c.DC, P).T).astype(f32)
    b1_a = np.ascontiguousarray(np.asarray(b1[-1]).reshape(c.MC, P).T).astype(f32)
    bv_a = np.ascontiguousarray(np.asarray(bv[-1])[None, :]).astype(bf)
    bout_a = np.ascontiguousarray(np.asarray(bout)[None, :]).astype(bf)

    hT_b = [_chunked(np.ascontiguousarray(h[b].T).astype(bf), c.DC)
            for b in range(B)]

    keys = np.arange(c.S).reshape(c.KC, P).T  # [ki, kc]
    groups_per_batch = n_cores // B
    in_maps = []
    for core in range(n_cores):
        b = core // groups_per_batch
        q0 = (core % groups_per_batch) * c.SQ
        m = (
            (q0 + np.arange(c.SQ))[None, None, :] >= keys[:, :, None]
        ).astype(bf)
        in_maps.append({
            "hT": hT_b[b],
            "hTq": np.ascontiguousarray(hT_b[b][:, :, q0 : q0 + c.SQ]),
            "wq": wq_a, "wk": wk_a, "wv": wv_a, "wo": wo_a, "w1": w1_a,
            "wout": wout_a,
            "mask": np.ascontiguousarray(m),
            "bqp": bq_a, "bkp": bk_a, "bop": bo_a, "b1p": b1_a,
            "bvr": bv_a, "boutr": bout_a,
        })
    return in_maps


def kernel(x, E, Wq, bq, Wk, bk, Wv, bv, Wo, bo, W1, b1, Wout, bout,
           num_heads=16, **kw):
    c = FULL
    assert int(num_heads) == c.H
    x = np.asarray(x)
    nc = _get_nc(c)
    in_maps = make_in_maps(x, E, Wq, bq, Wk, bk, Wv, bv, Wo, bo, W1, b1,
                           Wout, bout, cfg=c)
    try:
        res = run_bass_kernel_spmd(nc, in_maps, core_ids=list(range(8)))
    except Exception:
        # a previous session may have left a NeuronCore wedged
        # (NRT_EXEC_UNIT_UNRECOVERABLE); give the runtime time to reset
        # and retry once
        import time as _time
        _time.sleep(60)
        res = run_bass_kernel_spmd(nc, in_maps, core_ids=list(range(8)))
    B = x.shape[0]
    S = x.shape[1]
    out = np.empty((B, S, c.V), np.float32)
    groups_per_batch = 8 // B
    for core in range(8):
        b = core // groups_per_batch
        q0 = (core % groups_per_batch) * c.SQ
        out[b, q0 : q0 + c.SQ] = res.results[core]["out"]
    return out


# revision 22
# speedup vs baseline: 2.9936x; 2.9936x over previous
"""Trainium2 Bass kernel for a decoder-only transformer forward pass.

Reference semantics (see problem): the layer loop never feeds its output
back, so only the LAST layer's block matters:
    h   = E[x] + pos_encoding                         [B, S, D]
    Q/K/V = h @ Wq/k/v + b                            (last layer)
    att = softmax(causal(QK^T/sqrt(dk))) @ V
    h2  = att @ Wo + bo
    dec = tanh(h2 @ W1 + b1)                          [B, S, M]
    out = dec @ Wout + bout                           [B, S, OMEGA]

Sharding: 8 cores; core c handles batch b=c//4, query rows
[q0, q0+512), q0=(c%4)*512.  Every core computes K/V for its whole batch
(redundant x4) and its own 512-row slice of everything else, including
the dominant output-head matmul.  No collectives needed.

Device dataflow keeps activations TRANSPOSED (feature dim on partitions)
until the output head, so no on-device transposes are ever needed:
  hT -> QT/KT (lhsT = W in native layout), V (lhsT = hT)
  ST[k,q] = K.Q per head, P = exp(ST/sqrt(dk)) * causal_mask
  attT_unnorm[d',q] via lhsT=V; an appended ones-column in V yields the
  softmax denominator for free; normalize via reciprocal + ones-matmul
  partition-broadcast.
  h2T = Wo-matmul, decT = tanh(W1-matmul), logits = decT^T-matmul (the
  head flips back to q-on-partitions so the output DMA is contiguous).

All matmul operands are bf16 (fp32 PSUM accumulation).
"""

import sys

sys.path.insert(0, "/opt/trn_rl_repo")

import numpy as np
import ml_dtypes

import concourse.bass as bass  # noqa: F401
import concourse.mybir as mybir
import concourse.tile as tile
from concourse import bacc
from concourse.bass_utils import run_bass_kernel_spmd

P = 128
BF16 = mybir.dt.bfloat16
F32 = mybir.dt.float32


class Cfg:
    def __init__(self, S=2048, D=1024, DM=4096, V=16000, SQ=512, DK=64):
        self.S, self.D, self.DM, self.V, self.SQ, self.DK = S, D, DM, V, SQ, DK
        self.H = D // DK          # heads
        self.DC = D // P          # d chunks
        self.MC = DM // P         # ffn chunks
        self.KC = S // P          # key chunks
        self.QC = SQ // P         # q chunks


FULL = Cfg()


def _blocks(total, w=512):
    out = []
    v0 = 0
    while v0 < total:
        out.append((v0, min(w, total - v0)))
        v0 += w
    return out


def build_nc(cfg=FULL, debug=False, phases=4):
    c = cfg
    nc = bacc.Bacc("TRN2", target_bir_lowering=False, debug=debug)

    # weights grouped by OUTPUT chunk so they can be streamed per chunk:
    # wq[ki, out_ch, kc, kj] = Wq[kc*P+ki, out_ch*P+kj]
    hT = nc.dram_tensor("hT", [P, c.DC, c.S], BF16, kind="ExternalInput")
    hTq = nc.dram_tensor("hTq", [P, c.DC, c.SQ], BF16, kind="ExternalInput")
    wq = nc.dram_tensor("wq", [P, c.DC, c.DC, P], BF16, kind="ExternalInput")
    wk = nc.dram_tensor("wk", [P, c.DC, c.DC, P], BF16, kind="ExternalInput")
    wv = nc.dram_tensor("wv", [P, c.DC, c.D], BF16, kind="ExternalInput")
    wo = nc.dram_tensor("wo", [P, c.DC, c.D], BF16, kind="ExternalInput")
    w1 = nc.dram_tensor("w1", [P, c.MC, c.DC, P], BF16, kind="ExternalInput")
    wout = nc.dram_tensor("wout", [P, c.MC, c.V], BF16, kind="ExternalInput")
    mask = nc.dram_tensor("mask", [P, c.KC, c.SQ], BF16, kind="ExternalInput")
    bqp = nc.dram_tensor("bqp", [P, c.DC], F32, kind="ExternalInput")
    bkp = nc.dram_tensor("bkp", [P, c.DC], F32, kind="ExternalInput")
    bop = nc.dram_tensor("bop", [P, c.DC], F32, kind="ExternalInput")
    b1p = nc.dram_tensor("b1p", [P, c.MC], F32, kind="ExternalInput")
    bvr = nc.dram_tensor("bvr", [1, c.D], BF16, kind="ExternalInput")
    boutr = nc.dram_tensor("boutr", [1, c.V], BF16, kind="ExternalInput")
    out = nc.dram_tensor("out", [c.SQ, c.V], F32, kind="ExternalOutput")

    scale = 1.0 / np.sqrt(np.float32(c.DK))

    with tile.TileContext(nc) as tc:
        with (
            tc.tile_pool(name="const", bufs=1) as const,
            tc.tile_pool(name="persistB", bufs=1) as persistB,
        ):
            ones = const.tile([1, P], BF16, tag="ones")
            nc.any.memset(ones[:], 1.0)
            bq_sb = const.tile([P, c.DC], F32, tag="bq")
            bk_sb = const.tile([P, c.DC], F32, tag="bk")
            bo_sb = const.tile([P, c.DC], F32, tag="bo")
            b1_sb = const.tile([P, c.MC], F32, tag="b1")
            bv_sb = const.tile([1, c.D], BF16, tag="bv")
            nc.sync.dma_start(bq_sb[:], bqp[:])
            nc.sync.dma_start(bk_sb[:], bkp[:])
            nc.sync.dma_start(bo_sb[:], bop[:])
            nc.sync.dma_start(b1_sb[:], b1p[:])
            nc.sync.dma_start(bv_sb[:], bvr[:])

            attn_sb = persistB.tile([P, c.DC, c.SQ], BF16, tag="attn")
            dec_sb = persistB.tile([P, c.MC, c.SQ], BF16, tag="dec")

            with tc.tile_pool(name="persistA", bufs=1) as persistA:
                qt_sb = persistA.tile([P, c.DC, c.SQ], BF16, tag="qt")
                kt_sb = persistA.tile([P, c.DC, c.S], BF16, tag="kt")
                # V with an appended ones-column per head: [P, H, DK+1]
                v_sb = [
                    persistA.tile([P, c.H, c.DK + 1], BF16, tag=f"v{kk}",
                                  name=f"v{kk}")
                    for kk in range(c.KC)
                ]

                # ---------------- phase 1: projections ----------------
                with (
                    tc.tile_pool(name="wstream", bufs=3) as wstream,
                    tc.tile_pool(name="wvpool", bufs=1) as wvpool,
                    tc.tile_pool(name="hpool", bufs=1) as hpool,
                    tc.tile_pool(name="ps1", bufs=4, space="PSUM") as ps1,
                ):
                    wv_sb = wvpool.tile([P, c.DC, c.D], BF16, tag="wv")
                    hT_sb = hpool.tile([P, c.DC, c.S], BF16, tag="hT")
                    hTq_sb = hpool.tile([P, c.DC, c.SQ], BF16, tag="hTq")
                    # hTq gates the very first matmuls — issue it first
                    for ch in range(c.DC):
                        nc.sync.dma_start(hTq_sb[:, ch, :], hTq[:, ch, :])
                    for ch in range(c.DC):
                        nc.sync.dma_start(hT_sb[:, ch, :], hT[:, ch, :])
                    for ch in range(c.DC):
                        nc.sync.dma_start(wv_sb[:, ch, :], wv[:, ch, :])

                    # QT[d',q] = sum_d Wq[d,d'] hT[d,q]  (+bq)
                    for ch in range(c.DC):
                        wt = wstream.tile([P, c.DC, P], BF16, tag="wt")
                        nc.sync.dma_start(wt[:], wq[:, ch])
                        ps = ps1.tile([P, 512], F32, tag="ps1t")
                        for kc in range(c.DC):
                            nc.tensor.matmul(
                                ps[:, : c.SQ],
                                wt[:, kc, :],
                                hTq_sb[:, kc, :],
                                start=(kc == 0),
                                stop=(kc == c.DC - 1),
                            )
                        nc.vector.tensor_scalar_add(
                            qt_sb[:, ch, :], ps[:, : c.SQ],
                            bq_sb[:, ch : ch + 1],
                        )

                    # KT[d',k] over all S keys (+bk)
                    for ch in range(c.DC):
                        wt = wstream.tile([P, c.DC, P], BF16, tag="wt")
                        nc.sync.dma_start(wt[:], wk[:, ch])
                        for (n0, W) in _blocks(c.S):
                            ps = ps1.tile([P, 512], F32, tag="ps1t")
                            for kc in range(c.DC):
                                nc.tensor.matmul(
                                    ps[:, :W],
                                    wt[:, kc, :],
                                    hT_sb[:, kc, n0 : n0 + W],
                                    start=(kc == 0),
                                    stop=(kc == c.DC - 1),
                                )
                            nc.vector.tensor_scalar_add(
                                kt_sb[:, ch, n0 : n0 + W],
                                ps[:, :W],
                                bk_sb[:, ch : ch + 1],
                            )

                    # V[k,d'] native; lhsT = hT key-chunk, rhs = Wv
                    # (+bv via ones-row matmul on the free axis)
                    for kk in range(c.KC):
                        nc.any.memset(v_sb[kk][:, :, c.DK : c.DK + 1], 1.0)
                        for (d0, W) in _blocks(c.D):
                            ps = ps1.tile([P, 512], F32, tag="ps1t")
                            for kc in range(c.DC):
                                nc.tensor.matmul(
                                    ps[:, :W],
                                    hT_sb[:, kc, kk * P : (kk + 1) * P],
                                    wv_sb[:, kc, d0 : d0 + W],
                                    start=(kc == 0),
                                    stop=False,
                                )
                            nc.tensor.matmul(
                                ps[:, :W],
                                ones[0:1, :],
                                bv_sb[0:1, d0 : d0 + W],
                                start=False,
                                stop=True,
                            )
                            h0 = d0 // c.DK
                            h1 = (d0 + W) // c.DK
                            nc.vector.tensor_copy(
                                v_sb[kk][:, h0:h1, 0 : c.DK],
                                ps[:, :W].rearrange("p (h d) -> p h d",
                                                    d=c.DK),
                            )

                # ---------------- phase 2: attention ----------------
                with (
                    tc.tile_pool(name="maskp", bufs=1) as maskp,
                    tc.tile_pool(name="ppool", bufs=8) as ppool,
                    tc.tile_pool(name="npool", bufs=4) as npool,
                    tc.tile_pool(name="st_ps", bufs=5, space="PSUM") as st_ps,
                    tc.tile_pool(name="at_ps", bufs=2, space="PSUM") as at_ps,
                    tc.tile_pool(name="rb_ps", bufs=1, space="PSUM") as rb_ps,
                ):
                    mask_sb = maskp.tile([P, c.KC, c.SQ], BF16, tag="mask")
                    for kk in range(c.KC):
                        nc.sync.dma_start(mask_sb[:, kk, :], mask[:, kk, :])

                    for h in range(c.H if phases >= 2 else 0):
                        hp = 64 * (h % 2)
                        ch = h // 2
                        at = at_ps.tile([P, c.SQ], F32, tag="at")
                        p_tiles = []
                        for kk in range(c.KC):
                            st = st_ps.tile([P, c.SQ], F32, tag="st")
                            nc.tensor.matmul(
                                st[:],
                                kt_sb[hp : hp + c.DK, ch,
                                      kk * P : (kk + 1) * P],
                                qt_sb[hp : hp + c.DK, ch, :],
                                start=True,
                                stop=True,
                            )
                            p = ppool.tile([P, c.SQ], BF16, tag="p")
                            nc.scalar.activation(
                                p[:], st[:],
                                mybir.ActivationFunctionType.Exp,
                                scale=float(scale),
                            )
                            nc.vector.tensor_tensor(
                                p[:], p[:], mask_sb[:, kk, :],
                                mybir.AluOpType.mult,
                            )
                            p_tiles.append(p)
                        for kk in range(c.KC):
                            nc.tensor.matmul(
                                at[0 : c.DK + 1, :],
                                v_sb[kk][:, h, :],
                                p_tiles[kk][:],
                                start=(kk == 0),
                                stop=(kk == c.KC - 1),
                            )
                        # normalize by the ones-column sum (row DK of at)
                        r32 = npool.tile([1, c.SQ], F32, tag="r32")
                        nc.vector.reciprocal(r32[:], at[c.DK : c.DK + 1, :])
                        rbf = npool.tile([1, c.SQ], BF16, tag="rbf")
                        nc.vector.tensor_copy(rbf[:], r32[:])
                        rb = rb_ps.tile([c.DK, c.SQ], F32, tag="rb")
                        nc.tensor.matmul(
                            rb[:], ones[0:1, 0 : c.DK], rbf[:],
                            start=True, stop=True,
                        )
                        # rb to SBUF on DVE — on ACT it queues behind the next
                        # head's 16 exps (FIFO), holding the at-accumulator
                        # and stalling the PE on at-pool slots
                        rb_sb = npool.tile([c.DK, c.SQ], F32, tag="rb_sb")
                        nc.vector.tensor_copy(rb_sb[:], rb[:])
                        nc.vector.tensor_tensor(
                            attn_sb[hp : hp + c.DK, ch, :],
                            at[0 : c.DK, :],
                            rb_sb[:],
                            mybir.AluOpType.mult,
                        )

            # ------- phases 3+4 share one scope: Wo + FFN + output head ----
            # (lets the first Wout slab DMAs and head matmuls overlap the
            # FFN tail instead of waiting for a pool boundary)
            with (
                tc.tile_pool(name="wopool", bufs=1) as wopool,
                tc.tile_pool(name="w1stream", bufs=3) as w1stream,
                tc.tile_pool(name="h2pool", bufs=1) as h2pool,
                tc.tile_pool(name="ps3", bufs=3, space="PSUM") as ps3,
                tc.tile_pool(name="wout_p", bufs=3) as wout_p,
                tc.tile_pool(name="bout_p", bufs=2) as bout_p,
                tc.tile_pool(name="out_p", bufs=4) as out_p,
                tc.tile_pool(name="ps4", bufs=3, space="PSUM") as ps4,
            ):
                wo_sb = wopool.tile([P, c.DC, c.D], BF16, tag="wo")
                for ch in range(c.DC):
                    nc.sync.dma_start(wo_sb[:, ch, :], wo[:, ch, :])
                h2_sb = h2pool.tile([P, c.DC, c.SQ], BF16, tag="h2")
                for ch in range(c.DC if phases >= 3 else 0):
                    ps = ps3.tile([P, 512], F32, tag="ps3t")
                    for kc in range(c.DC):
                        nc.tensor.matmul(
                            ps[:, : c.SQ],
                            wo_sb[:, kc, ch * P : (ch + 1) * P],
                            attn_sb[:, kc, :],
                            start=(kc == 0),
                            stop=(kc == c.DC - 1),
                        )
                    nc.vector.tensor_scalar_add(
                        h2_sb[:, ch, :], ps[:, : c.SQ], bo_sb[:, ch : ch + 1]
                    )
                for mch in range(c.MC if phases >= 3 else 0):
                    wt = w1stream.tile([P, c.DC, P], BF16, tag="w1t")
                    nc.sync.dma_start(wt[:], w1[:, mch])
                    ps = ps3.tile([P, 512], F32, tag="ps3t")
                    for kc in range(c.DC):
                        nc.tensor.matmul(
                            ps[:, : c.SQ],
                            wt[:, kc, :],
                            h2_sb[:, kc, :],
                            start=(kc == 0),
                            stop=(kc == c.DC - 1),
                        )
                    nc.scalar.activation(
                        dec_sb[:, mch, :], ps[:, : c.SQ],
                        mybir.ActivationFunctionType.Tanh,
                        bias=b1_sb[:, mch : mch + 1],
                        scale=1.0,
                    )

                # ---------------- phase 4: output head ----------------
                for (v0, W) in (_blocks(c.V) if phases >= 4 else []):
                    wt = wout_p.tile([P, c.MC, 512], BF16, tag="wt4")
                    step = max(1, c.MC // 4)
                    for m0 in range(0, c.MC, step):
                        m1 = min(c.MC, m0 + step)
                        nc.sync.dma_start(
                            wt[:, m0:m1, :W], wout[:, m0:m1, v0 : v0 + W]
                        )
                    bt = bout_p.tile([1, 512], BF16, tag="bt")
                    nc.sync.dma_start(bt[0:1, :W], boutr[0:1, v0 : v0 + W])
                    # broadcast bout across partitions once per v-block;
                    # the bias is then added during psum eviction on DVE
                    bb = ps4.tile([P, 512], F32, tag="bb", bufs=1)
                    nc.tensor.matmul(
                        bb[:, :W], ones[0:1, :], bt[0:1, :W],
                        start=True, stop=True,
                    )
                    bb_sb = bout_p.tile([P, 512], BF16, tag="bb_sb")
                    nc.scalar.activation(
                        bb_sb[:, :W], bb[:, :W],
                        mybir.ActivationFunctionType.Copy,
                    )
                    for qc in range(c.QC):
                        ps = ps4.tile([P, 512], F32, tag="ps4t")
                        for mch in range(c.MC):
                            nc.tensor.matmul(
                                ps[:, :W],
                                dec_sb[:, mch, qc * P : (qc + 1) * P],
                                wt[:, mch, :W],
                                start=(mch == 0),
                                stop=(mch == c.MC - 1),
                            )
                        ot = out_p.tile([P, 512], F32, tag="ot")
                        nc.vector.tensor_tensor(
                            ot[:, :W], ps[:, :W], bb_sb[:, :W],
                            mybir.AluOpType.add,
                        )
                        nc.sync.dma_start(
                            out[qc * P : (qc + 1) * P, v0 : v0 + W],
                            ot[:, :W],
                        )

    nc.compile()
    return nc


# ---------------------------------------------------------------------------
# host side
# ---------------------------------------------------------------------------

def _pos_encoding(seq_len, d):
    pos = np.arange(seq_len, dtype=np.float32)[:, None]
    div = np.exp(
        np.arange(0, d, 2, dtype=np.float32) * (-np.log(10000.0) / d)
    )
    pe = np.zeros((seq_len, d), dtype=np.float32)
    pe[:, 0::2] = np.sin(pos * div)
    pe[:, 1::2] = np.cos(pos * div)
    return pe


def _chunked(a, pdim_chunks):
    """[N, F] -> [128, N//128, F] with row n = ko*128+ki -> [ki, ko, f]."""
    n, f = a.shape
    return np.ascontiguousarray(
        a.reshape(pdim_chunks, P, f).transpose(1, 0, 2)
    )


def _grouped(a, kchunks, ochunks):
    """[K, O] -> [128, O//128, K//128, 128]: [ki, oc, kc, kj]."""
    k, o = a.shape
    return np.ascontiguousarray(
        a.reshape(kchunks, P, ochunks, P).transpose(1, 2, 0, 3)
    )


_NC_CACHE = {}


def _get_nc(cfg=FULL):
    key = (cfg.S, cfg.D, cfg.DM, cfg.V, cfg.SQ)
    if key not in _NC_CACHE:
        _NC_CACHE[key] = build_nc(cfg)
    return _NC_CACHE[key]


def make_in_maps(x, E, Wq, bq, Wk, bk, Wv, bv, Wo, bo, W1, b1, Wout, bout,
                 cfg=FULL, n_cores=8):
    c = cfg
    bf = ml_dtypes.bfloat16
    x = np.asarray(x)
    E = np.asarray(E, dtype=np.float32)
    B = x.shape[0]
    h = E[x] + _pos_encoding(x.shape[1], E.shape[1])[None]

    wq_a = _grouped(np.asarray(Wq[-1]).astype(bf), c.DC, c.DC)
    wk_a = _grouped(np.asarray(Wk[-1]).astype(bf), c.DC, c.DC)
    wv_a = _chunked(np.asarray(Wv[-1]).astype(bf), c.DC)
    wo_a = _chunked(np.asarray(Wo[-1]).astype(bf), c.DC)
    w1_a = _grouped(np.asarray(W1[-1]).astype(bf), c.DC, c.MC)
    wout_a = _chunked(np.asarray(Wout).astype(bf), c.MC)
    f32 = np.float32
    bq_a = np.ascontiguousarray(np.asarray(bq[-1]).reshape(c.DC, P).T).astype(f32)
    bk_a = np.ascontiguousarray(np.asarray(bk[-1]).reshape(c.DC, P).T).astype(f32)
    bo_a = np.ascontiguousarray(np.asarray(bo[-1]).reshape(c.DC, P).T).astype(f32)
    b1_a = np.ascontiguousarray(np.asarray(b1[-1]).reshape(c.MC, P).T).astype(f32)
    bv_a = np.ascontiguousarray(np.asarray(bv[-1])[None, :]).astype(bf)
    bout_a = np.ascontiguousarray(np.asarray(bout)[None, :]).astype(bf)

    hT_b = [_chunked(np.ascontiguousarray(h[b].T).astype(bf), c.DC)
            for b in range(B)]

    keys = np.arange(c.S).reshape(c.KC, P).T  # [ki, kc]
    groups_per_batch = n_cores // B
    in_maps = []
    for core in range(n_cores):
        b = core // groups_per_batch
        q0 = (core % groups_per_batch) * c.SQ
        m = (
            (q0 + np.arange(c.SQ))[None, None, :] >= keys[:, :, None]
        ).astype(bf)
        in_maps.append({
            "hT": hT_b[b],
            "hTq": np.ascontiguousarray(hT_b[b][:, :, q0 : q0 + c.SQ]),
            "wq": wq_a, "wk": wk_a, "wv": wv_a, "wo": wo_a, "w1": w1_a,
            "wout": wout_a,
            "mask": np.ascontiguousarray(m),
            "bqp": bq_a, "bkp": bk_a, "bop": bo_a, "b1p": b1_a,
            "bvr": bv_a, "boutr": bout_a,
        })
    return in_maps


def kernel(x, E, Wq, bq, Wk, bk, Wv, bv, Wo, bo, W1, b1, Wout, bout,
           num_heads=16, **kw):
    c = FULL
    assert int(num_heads) == c.H
    x = np.asarray(x)
    nc = _get_nc(c)
    in_maps = make_in_maps(x, E, Wq, bq, Wk, bk, Wv, bv, Wo, bo, W1, b1,
                           Wout, bout, cfg=c)
    try:
        res = run_bass_kernel_spmd(nc, in_maps, core_ids=list(range(8)))
    except Exception:
        # a previous session may have left a NeuronCore wedged
        # (NRT_EXEC_UNIT_UNRECOVERABLE); give the runtime time to reset
        # and retry once
        import time as _time
        _time.sleep(60)
        res = run_bass_kernel_spmd(nc, in_maps, core_ids=list(range(8)))
    B = x.shape[0]
    S = x.shape[1]
    out = np.empty((B, S, c.V), np.float32)
    groups_per_batch = 8 // B
    for core in range(8):
        b = core // groups_per_batch
        q0 = (core % groups_per_batch) * c.SQ
        out[b, q0 : q0 + c.SQ] = res.results[core]["out"]
    return out
